# revision 10
# baseline (speedup 1.0000x reference)
"""NSMCell message-passing kernel for 8 Trainium2 NeuronCores.

Contract: kernel(**inputs) takes the FULL unsharded inputs (numpy/jax arrays)
and returns the FULL (N,) float32 output, matching reference.reference().

Math restructuring (exact, up to float assoc.):
  edge path:  msg @ w_rel = segment_sum(dist[src] * (elu((i_b*a_e)@W_edge) @ w_rel), dst)
              and (i_b (*) a_e) @ W_edge = a_e @ (diag(i_b) @ W_edge) = a_e @ U_b
    -> per-edge scalar t_e = w_rel . elu(a_e @ U_b),  b = edge_batch_indices[e]
  node path:  s_n = w_node . elu(sum_p attr[n,p] @ V_{b,p}),
              V_{b,p} = sim[b,p] * diag(i_b) @ W_props[p],  b = node_indices[n]
  host epilogue (O(N+E) scalar work): scatter-add t into nodes by dst,
  two segment softmaxes over graphs, final mix by relation_similarity.

Device work = all the heavy lifting (the E*H*H and N*P*H*H matmuls + elu over
the (E,H)/(N,H) intermediates), sharded: edges sorted by graph id and dealt
8 ways; nodes (already sorted by graph) dealt 8 ways. Single SPMD program.

elu on device: ACT computes e = exp(z) (f32 internal, from PSUM); a custom
DVE op computes elu(z) = relu(z) + min(e, 1) - 1 in one fused pass.
"""

import os
import sys
import types

import numpy as np

# ---------------------------------------------------------------------------
# problem constants (hardcoded per contract)
N, P, H, E, B = 100000, 4, 128, 1000000, 64
NCORES = 8
TZ = 512          # matmul moving-dim tile (one PSUM bank of f32)
ZCOLS = 1024      # z PSUM tile free size (2 banks) = 2 matmuls
TACC_COLS = 512   # t-accumulator PSUM tile (1 bank)

_DT = os.environ.get("KERNEL_DTYPE", "fp16")  # fp16 | fp32


# ---------------------------------------------------------------------------
def _install_ntff_hook():
    """Allow BASS_TRACE=1 profiling under axon (test.py); harmless otherwise."""
    try:
        from antenv.axon_hooks import get_axon_ntff_profile_hook  # noqa: F401
        return
    except ImportError:
        pass
    try:
        from trn_agent_boot.trn_boot import _ntff_profile_via_ctypes
        hook = _ntff_profile_via_ctypes("/opt/axon/libaxon_pjrt.so")
    except Exception:
        hook = None
    mod = types.ModuleType("antenv.axon_hooks")
    _state = {"hook": hook}
    mod.get_axon_ntff_profile_hook = lambda: _state["hook"]
    mod.set_axon_ntff_profile_hook = lambda h: _state.__setitem__("hook", h)
    sys.modules["antenv.axon_hooks"] = mod
    try:
        import antenv
        antenv.axon_hooks = mod
    except ImportError:
        pass


def _make_elu_op():
    """Register custom DVE op: out = relu(in0) + min(in1, 1) - 1 (= elu when
    in1 == exp(in0)). Runtime registration: append to dve_ops.OPS."""
    from concourse import dve_ops
    from concourse.dve_spec import Spec, Src0, Src1, One, relu, minn, lower
    from concourse.dve_uop import DveOpSpec

    name = "ELU_FROM_EXP_ANT"
    for op in dve_ops.OPS:
        if op.name == name:
            return op
    spec = Spec(
        body=relu(Src0) + minn(Src1, One) - One,
        reference=lambda in0, in1, s0, s1, imm2: (
            np.maximum(np.nan_to_num(in0, nan=0.0), 0)
            + np.minimum(in1, np.float32(1.0))
            - np.float32(1.0)
        ).astype(np.float32),
    )
    row = dve_ops._CUSTOM_DVE_ROW_BASE + len(dve_ops.OPS)
    assert row < 0x20
    shas = {}
    for ver in ("v3", "v4"):
        shas[ver] = DveOpSpec(
            name=name, opcode=row, uops=lower(spec, ver=ver), rd1_en=True
        ).sha(ver)
    op = dve_ops.DveOp(name, spec, subdim=False, uops_sha=shas)
    dve_ops.OPS.append(op)
    dve_ops.CUSTOM_DVE_SPECS[name] = spec
    dve_ops._SUB_OPCODE_FOR_NAME[name] = row
    return op


# ---------------------------------------------------------------------------
def _build_program(me: int, mn: int, dt_lo, np_lo):
    """Build the SPMD bass program. me = per-(core,graph) padded edge count
    (multiple of ZCOLS); mn = per-(core,graph) padded node count (<= TZ)."""
    import concourse.tile as tile
    from concourse import bacc
    import concourse.mybir as mybir

    f32 = mybir.dt.float32
    Exp = mybir.ActivationFunctionType.Exp
    elu_op = _make_elu_op()

    ncol_t = B * me // H                  # t output columns (128 rows each)
    nchunk_s = (mn + H - 1) // H          # node chunks per graph (cols per b)
    ncol_s = B * nchunk_s

    nc = bacc.Bacc("TRN2", target_bir_lowering=False, debug=False,
                   num_devices=NCORES)

    ea_in = nc.dram_tensor("ea_t", [B, me // TZ, H, TZ], dt_lo,
                           kind="ExternalInput")
    na_in = nc.dram_tensor("na_t", [B, P, H, mn], dt_lo, kind="ExternalInput")
    u_in = nc.dram_tensor("u_tab", [B, H, H], dt_lo, kind="ExternalInput")
    v_in = nc.dram_tensor("v_tab", [B * P, H, H], dt_lo, kind="ExternalInput")
    wr_in = nc.dram_tensor("w_rel", [H, 1], dt_lo, kind="ExternalInput")
    wn_in = nc.dram_tensor("w_node", [H, 1], dt_lo, kind="ExternalInput")
    t_out = nc.dram_tensor("t_out", [H, ncol_t], f32, kind="ExternalOutput")
    s_out = nc.dram_tensor("s_out", [H, ncol_s], f32, kind="ExternalOutput")

    with tile.TileContext(nc) as tc:
        with (
            tc.tile_pool(name="consts", bufs=1) as cpool,
            tc.tile_pool(name="ework", bufs=3) as epool,
            tc.tile_pool(name="nwork", bufs=3) as npool,
            tc.tile_pool(name="outs", bufs=2) as opool,
            tc.tile_pool(name="zpsum", bufs=2, space="PSUM") as zpool,
            tc.tile_pool(name="tpsum", bufs=2, space="PSUM") as tpool,
            tc.tile_pool(name="znpsum", bufs=1, space="PSUM") as znpool,
            tc.tile_pool(name="spsum", bufs=1, space="PSUM") as spool,
        ):
            u_sb = cpool.tile([H, B, H], dt_lo)
            nc.sync.dma_start(u_sb[:], u_in.ap().rearrange("b h k -> h b k"))
            v_sb = cpool.tile([H, B * P, H], dt_lo)
            nc.sync.dma_start(v_sb[:], v_in.ap().rearrange("g h k -> h g k"))
            wr_sb = cpool.tile([H, 1], dt_lo)
            nc.sync.dma_start(wr_sb[:], wr_in.ap())
            wn_sb = cpool.tile([H, 1], dt_lo)
            nc.sync.dma_start(wn_sb[:], wn_in.ap())

            # ---------------- edge phase ----------------
            tcol = 0
            tacc = tpool.tile([H, TACC_COLS], f32, tag="tacc")
            for b in range(B):
                ea_sb = epool.tile([H, me // TZ, TZ], dt_lo, tag="ea")
                nc.sync.dma_start(
                    ea_sb[:], ea_in.ap()[b].rearrange("j h t -> h j t")
                )
                for half in range(me // ZCOLS):
                    z = zpool.tile([H, ZCOLS], f32, tag="z")
                    for q in range(ZCOLS // TZ):
                        nc.tensor.matmul(
                            z[:, q * TZ:(q + 1) * TZ],
                            u_sb[:, b, :],
                            ea_sb[:, half * (ZCOLS // TZ) + q, :],
                            start=True, stop=True,
                        )
                    e_sb = epool.tile([H, ZCOLS], dt_lo, tag="e")
                    nc.scalar.activation(e_sb[:], z[:], Exp)
                    psi = epool.tile([H, ZCOLS], dt_lo, tag="psi")
                    nc.vector._custom_dve(elu_op, out=psi[:], in0=z[:],
                                          in1=e_sb[:])
                    for c in range(ZCOLS // H):
                        col = tcol % TACC_COLS
                        nc.tensor.matmul(
                            tacc[:, col:col + 1],
                            psi[:, c * H:(c + 1) * H],
                            wr_sb[:],
                            start=True, stop=True,
                        )
                        tcol += 1
                        if tcol % TACC_COLS == 0:
                            t_sb = opool.tile([H, TACC_COLS], f32, tag="tsb")
                            nc.vector.tensor_copy(t_sb[:], tacc[:])
                            lo = tcol - TACC_COLS
                            nc.sync.dma_start(t_out.ap()[:, lo:tcol], t_sb[:])
                            if tcol < ncol_t:
                                tacc = tpool.tile([H, TACC_COLS], f32,
                                                  tag="tacc")
            if tcol % TACC_COLS != 0:
                t_sb = opool.tile([H, TACC_COLS], f32, tag="tsb")
                rem = tcol % TACC_COLS
                nc.vector.tensor_copy(t_sb[:, :rem], tacc[:, :rem])
                nc.sync.dma_start(t_out.ap()[:, tcol - rem:tcol],
                                  t_sb[:, :rem])

            # ---------------- node phase ----------------
            sacc = spool.tile([H, ncol_s], f32)
            for b in range(B):
                at_sb = npool.tile([H, P, mn], dt_lo, tag="nat")
                nc.sync.dma_start(
                    at_sb[:], na_in.ap()[b].rearrange("p h m -> h p m")
                )
                zn = znpool.tile([H, mn], f32, tag="zn")
                for p in range(P):
                    nc.tensor.matmul(
                        zn[:],
                        v_sb[:, b * P + p, :],
                        at_sb[:, p, :],
                        start=(p == 0), stop=(p == P - 1),
                    )
                en = npool.tile([H, mn], dt_lo, tag="en")
                nc.scalar.activation(en[:], zn[:], Exp)
                psn = npool.tile([H, mn], dt_lo, tag="psn")
                nc.vector._custom_dve(elu_op, out=psn[:], in0=zn[:], in1=en[:])
                for c in range(nchunk_s):
                    w = min(H, mn - c * H)
                    nc.tensor.matmul(
                        sacc[:w, b * nchunk_s + c: b * nchunk_s + c + 1],
                        psn[:, c * H:c * H + w],
                        wn_sb[:],
                        start=True, stop=True,
                    )
            s_sb = opool.tile([H, ncol_s], f32, tag="ssb")
            nc.vector.tensor_copy(s_sb[:], sacc[:])
            nc.sync.dma_start(s_out.ap()[:], s_sb[:])

    nc.compile()
    return nc


# ---------------------------------------------------------------------------
def kernel(node_attrs, edge_attrs, instruction_batch, distribution,
           node_prop_similarities, relation_similarity,
           W_props, W_edge, w_node_score, w_rel_score,
           edge_indices, node_indices, edge_batch_indices):
    _install_ntff_hook()
    from concourse import bass_utils

    np_lo = np.float16 if _DT == "fp16" else np.float32

    na = np.asarray(node_attrs, np.float32)
    ea = np.asarray(edge_attrs, np.float32)
    ib = np.asarray(instruction_batch, np.float32)
    dist = np.asarray(distribution, np.float32)
    nps = np.asarray(node_prop_similarities, np.float32)
    rs = np.asarray(relation_similarity, np.float32)
    Wp = np.asarray(W_props, np.float32)
    We = np.asarray(W_edge, np.float32)
    wn = np.asarray(w_node_score, np.float32)
    wr = np.asarray(w_rel_score, np.float32)
    ei = np.asarray(edge_indices).astype(np.int64)
    ni = np.asarray(node_indices).astype(np.int64)
    ebi = np.asarray(edge_batch_indices).astype(np.int64)
    src, dst = ei[0], ei[1]

    # ---- transformed weight tables (host, exact f32 then cast) ----
    U = ib[:, :, None] * We[None, :, :]                        # (B,H,H)
    V = (nps[:, :, None, None] * ib[:, None, :, None] *
         Wp[None, :, :, :])                                    # (B,P,H,H)
    U_lo = U.astype(np_lo)
    V_lo = V.reshape(B * P, H, H).astype(np_lo)
    wr_lo = wr.reshape(H, 1).astype(np_lo)
    wn_lo = wn.reshape(H, 1).astype(np_lo)

    # ---- edge sharding: sort by graph, pad groups, deal 8 ways ----
    order = np.argsort(ebi, kind="stable")
    ecounts = np.bincount(ebi, minlength=B)
    estarts = np.concatenate([[0], np.cumsum(ecounts)[:-1]])
    me = ((int(ecounts.max()) + NCORES * ZCOLS - 1)
          // (NCORES * ZCOLS)) * ZCOLS                         # per (core,b)
    assert me % ZCOLS == 0 and ecounts.max() <= NCORES * me
    ea_lo = ea.astype(np_lo)
    ebuf = np.zeros((B, NCORES * me, H), np_lo)
    for b in range(B):
        s, c = estarts[b], ecounts[b]
        ebuf[b, :c] = ea_lo[order[s:s + c]]
    # (B, cores, j, TZ, H) -> (cores, B, j, H, TZ)
    ea_t = np.ascontiguousarray(
        ebuf.reshape(B, NCORES, me // TZ, TZ, H).transpose(1, 0, 2, 4, 3)
    )
    del ebuf

    # ---- node sharding: already sorted by graph ----
    ncounts = np.bincount(ni, minlength=B)
    nstarts = np.concatenate([[0], np.cumsum(ncounts)[:-1]])
    mn = ((int(ncounts.max()) + NCORES - 1) // NCORES + 1) // 2 * 2
    mn = min(max(mn, 2), TZ)
    assert ncounts.max() <= NCORES * mn
    na_lo = na.astype(np_lo)
    nbuf = np.zeros((B, NCORES * mn, P, H), np_lo)
    for b in range(B):
        s, c = nstarts[b], ncounts[b]
        nbuf[b, :c] = na_lo[s:s + c]
    # (B, cores, mn, P, H) -> (cores, B, P, H, mn)
    na_t = np.ascontiguousarray(
        nbuf.reshape(B, NCORES, mn, P, H).transpose(1, 0, 3, 4, 2)
    )
    del nbuf

    import concourse.mybir as mybir
    dt_lo = mybir.dt.float16 if np_lo is np.float16 else mybir.dt.float32

    nc = _build_program(me, mn, dt_lo, np_lo)

    in_maps = []
    for c in range(NCORES):
        in_maps.append({
            "ea_t": ea_t[c],
            "na_t": na_t[c],
            "u_tab": U_lo,
            "v_tab": V_lo,
            "w_rel": wr_lo,
            "w_node": wn_lo,
        })

    res = bass_utils.run_bass_kernel_spmd(
        nc, in_maps, core_ids=list(range(NCORES)),
        trace=bool(os.environ.get("BASS_TRACE")),
        tmpdir=os.environ.get("KERNEL_TRACE_DIR") or None,
    )
    kernel.last_results = res  # for test.py profiling introspection

    # ---- host epilogue ----
    # t mapping: t_dev[c][p, col], col = b*(me//H) + cc, edge pos in (c,b)
    # chunk = cc*H + p; global pos in sorted group = c*me + cc*H + p.
    ncol_t = B * me // H
    cpg = me // H                                    # cols per graph
    t_dev = np.stack([np.asarray(res.results[c]["t_out"])
                      for c in range(NCORES)])       # (8, H, ncol_t)
    col = np.arange(ncol_t)
    b_of_col = col // cpg
    cc_of_col = col % cpg
    # pos[c,p,col] = c*me + cc*H + p
    pos = (np.arange(NCORES)[:, None, None] * me
           + cc_of_col[None, None, :] * H
           + np.arange(H)[None, :, None])            # (8,H,ncol_t)
    bgrid = np.broadcast_to(b_of_col[None, None, :], pos.shape)
    valid = pos < ecounts[bgrid]
    sorted_idx = estarts[bgrid] + pos                # index into `order`
    t_full = np.zeros(E, np.float64)
    t_full[order[sorted_idx[valid]]] = t_dev[valid]

    # s mapping: col = b*nchunk_s + chunk; node pos = c*mn + chunk*H + p
    nchunk_s = (mn + H - 1) // H
    ncol_s = B * nchunk_s
    s_dev = np.stack([np.asarray(res.results[c]["s_out"])
                      for c in range(NCORES)])       # (8, H, ncol_s)
    scol = np.arange(ncol_s)
    b_of_scol = scol // nchunk_s
    ch_of_scol = scol % nchunk_s
    spos = (np.arange(NCORES)[:, None, None] * mn
            + ch_of_scol[None, None, :] * H
            + np.arange(H)[None, :, None])
    sbgrid = np.broadcast_to(b_of_scol[None, None, :], spos.shape)
    svalid = (spos < ncounts[sbgrid]) & ((ch_of_scol[None, None, :] * H
              + np.arange(H)[None, :, None]) < mn)
    s_full = np.zeros(N, np.float64)
    s_full[nstarts[sbgrid[svalid]] + spos[svalid]] = s_dev[svalid]

    # scatter-add edge scalars into nodes, then segment softmaxes
    acc = np.bincount(dst, weights=dist[src].astype(np.float64) * t_full,
                      minlength=N)

    def seg_softmax(x):
        m = np.full(B, -np.inf)
        np.maximum.at(m, ni, x)
        e = np.exp(x - m[ni])
        ssum = np.zeros(B, np.float64)
        np.add.at(ssum, ni, e)
        return e / ssum[ni]

    next_rel = seg_softmax(acc)
    next_states = seg_softmax(s_full)
    rsn = rs[ni].astype(np.float64)
    out = rsn * next_rel + (1.0 - rsn) * next_states
    return out.astype(np.float32)


# revision 18
# speedup vs baseline: 1.0817x; 1.0817x over previous
"""NSMCell message-passing kernel for 8 Trainium2 NeuronCores.

Contract: kernel(**inputs) takes the FULL unsharded inputs (numpy/jax arrays)
and returns the FULL (N,) float32 output, matching reference.reference().

Math restructuring (exact, up to float assoc.):
  edge path:  msg @ w_rel = segment_sum(dist[src] * (elu((i_b*a_e)@W_edge) @ w_rel), dst)
              and (i_b (*) a_e) @ W_edge = a_e @ (diag(i_b) @ W_edge) = a_e @ U_b
    -> per-edge scalar t_e = w_rel . elu(a_e @ U_b),  b = edge_batch_indices[e]
  node path:  s_n = w_node . elu(sum_p attr[n,p] @ V_{b,p}),
              V_{b,p} = sim[b,p] * diag(i_b) @ W_props[p],  b = node_indices[n]
  host epilogue (O(N+E) scalar work): scatter-add t into nodes by dst,
  two segment softmaxes over graphs, final mix by relation_similarity.

Device work = all the heavy lifting (the E*H*H and N*P*H*H matmuls + elu over
the (E,H)/(N,H) intermediates), sharded: edges sorted by graph id and dealt
8 ways; nodes (already sorted by graph) dealt 8 ways. Single SPMD program.

elu on device: ACT computes e = exp(z) (f32 internal, from PSUM); a custom
DVE op computes elu(z) = relu(z) + min(e, 1) - 1 in one fused pass.
"""

import os
import sys
import types

import numpy as np

# ---------------------------------------------------------------------------
# problem constants (hardcoded per contract)
N, P, H, E, B = 100000, 4, 128, 1000000, 64
NCORES = 8
TZ = 512          # matmul moving-dim tile (one PSUM bank of f32)
ZCOLS = 1024      # z PSUM tile free size (2 banks) = 2 matmuls
TACC_COLS = 512   # t-accumulator PSUM tile (1 bank)

_DT = os.environ.get("KERNEL_DTYPE", "fp16")  # fp16 | fp32


# ---------------------------------------------------------------------------
def _install_ntff_hook():
    """Allow BASS_TRACE=1 profiling under axon (test.py); harmless otherwise."""
    try:
        from antenv.axon_hooks import get_axon_ntff_profile_hook  # noqa: F401
        return
    except ImportError:
        pass
    try:
        from trn_agent_boot.trn_boot import _ntff_profile_via_ctypes
        hook = _ntff_profile_via_ctypes("/opt/axon/libaxon_pjrt.so")
    except Exception:
        hook = None
    mod = types.ModuleType("antenv.axon_hooks")
    _state = {"hook": hook}
    mod.get_axon_ntff_profile_hook = lambda: _state["hook"]
    mod.set_axon_ntff_profile_hook = lambda h: _state.__setitem__("hook", h)
    sys.modules["antenv.axon_hooks"] = mod
    try:
        import antenv
        antenv.axon_hooks = mod
    except ImportError:
        pass


def _make_elu_op():
    """Register custom DVE op: out = relu(in0) + min(in1, 1) - 1 (= elu when
    in1 == exp(in0)). Runtime registration: append to dve_ops.OPS."""
    from concourse import dve_ops
    from concourse.dve_spec import Spec, Src0, Src1, One, relu, minn, lower
    from concourse.dve_uop import DveOpSpec

    name = "ELU_FROM_EXP_ANT"
    for op in dve_ops.OPS:
        if op.name == name:
            return op
    spec = Spec(
        body=relu(Src0) + minn(Src1, One) - One,
        reference=lambda in0, in1, s0, s1, imm2: (
            np.maximum(np.nan_to_num(in0, nan=0.0), 0)
            + np.minimum(in1, np.float32(1.0))
            - np.float32(1.0)
        ).astype(np.float32),
    )
    row = dve_ops._CUSTOM_DVE_ROW_BASE + len(dve_ops.OPS)
    assert row < 0x20
    shas = {}
    for ver in ("v3", "v4"):
        shas[ver] = DveOpSpec(
            name=name, opcode=row, uops=lower(spec, ver=ver), rd1_en=True
        ).sha(ver)
    op = dve_ops.DveOp(name, spec, subdim=False, uops_sha=shas)
    dve_ops.OPS.append(op)
    dve_ops.CUSTOM_DVE_SPECS[name] = spec
    dve_ops._SUB_OPCODE_FOR_NAME[name] = row
    return op


# ---------------------------------------------------------------------------
def _build_program(me: int, mn: int, dt_lo, np_lo):
    """Build the SPMD bass program. me = per-(core,graph) padded edge count
    (multiple of ZCOLS); mn = per-(core,graph) padded node count (<= TZ)."""
    import concourse.tile as tile
    from concourse import bacc
    import concourse.mybir as mybir

    f32 = mybir.dt.float32
    Exp = mybir.ActivationFunctionType.Exp
    elu_op = _make_elu_op()

    ncol_t = B * me // H                  # t output columns (128 rows each)
    nchunk_s = (mn + H - 1) // H          # node chunks per graph (cols per b)
    ncol_s = B * nchunk_s

    nc = bacc.Bacc("TRN2", target_bir_lowering=False, debug=False,
                   num_devices=NCORES)

    # all DRAM layouts are pre-transposed on host so that every DMA reads
    # long contiguous runs per SBUF partition
    ea_in = nc.dram_tensor("ea_t", [B, H, me], dt_lo, kind="ExternalInput")
    na_in = nc.dram_tensor("na_t", [B, H, P, mn], dt_lo, kind="ExternalInput")
    u_in = nc.dram_tensor("u_tab", [H, B, H], dt_lo, kind="ExternalInput")
    v_in = nc.dram_tensor("v_tab", [H, B * P, H], dt_lo, kind="ExternalInput")
    wr_in = nc.dram_tensor("w_rel", [H, 1], dt_lo, kind="ExternalInput")
    wn_in = nc.dram_tensor("w_node", [H, 1], dt_lo, kind="ExternalInput")
    t_out = nc.dram_tensor("t_out", [H, ncol_t], f32, kind="ExternalOutput")
    s_out = nc.dram_tensor("s_out", [H, ncol_s], f32, kind="ExternalOutput")

    with tile.TileContext(nc) as tc:
        with (
            tc.tile_pool(name="consts", bufs=1) as cpool,
            tc.tile_pool(name="ework", bufs=3) as epool,
            tc.tile_pool(name="nwork", bufs=3) as npool,
            tc.tile_pool(name="outs", bufs=2) as opool,
            tc.tile_pool(name="zpsum", bufs=3, space="PSUM") as zpool,
            tc.tile_pool(name="tpsum", bufs=1, space="PSUM") as tpool,
            tc.tile_pool(name="spsum", bufs=1, space="PSUM") as spool,
        ):
            u_sb = cpool.tile([H, B, H], dt_lo)
            nc.sync.dma_start(u_sb[:], u_in.ap())
            v_sb = cpool.tile([H, B * P, H], dt_lo)
            nc.sync.dma_start(v_sb[:], v_in.ap())
            wr_sb = cpool.tile([H, 1], dt_lo)
            nc.sync.dma_start(wr_sb[:], wr_in.ap())
            wn_sb = cpool.tile([H, 1], dt_lo)
            nc.sync.dma_start(wn_sb[:], wn_in.ap())

            # ---------------- edge phase ----------------
            tcol = 0
            tacc = tpool.tile([H, TACC_COLS], f32, tag="tacc")
            for b in range(B):
                ea_sb = epool.tile([H, me], dt_lo, tag="ea")
                nc.sync.dma_start(ea_sb[:], ea_in.ap()[b])
                for half in range(me // ZCOLS):
                    z = zpool.tile([H, ZCOLS], f32, tag="z")
                    for q in range(ZCOLS // TZ):
                        lo = half * ZCOLS + q * TZ
                        nc.tensor.matmul(
                            z[:, q * TZ:(q + 1) * TZ],
                            u_sb[:, b, :],
                            ea_sb[:, lo:lo + TZ],
                            start=True, stop=True,
                        )
                    e_sb = epool.tile([H, ZCOLS], dt_lo, tag="e")
                    nc.scalar.activation(e_sb[:], z[:], Exp)
                    psi = epool.tile([H, ZCOLS], dt_lo, tag="psi")
                    nc.vector._custom_dve(elu_op, out=psi[:], in0=z[:],
                                          in1=e_sb[:])
                    for c in range(ZCOLS // H):
                        col = tcol % TACC_COLS
                        nc.tensor.matmul(
                            tacc[:, col:col + 1],
                            psi[:, c * H:(c + 1) * H],
                            wr_sb[:],
                            start=True, stop=True,
                        )
                        tcol += 1
                        if tcol % TACC_COLS == 0:
                            t_sb = opool.tile([H, TACC_COLS], f32, tag="tsb")
                            nc.vector.tensor_copy(t_sb[:], tacc[:])
                            lo = tcol - TACC_COLS
                            nc.sync.dma_start(t_out.ap()[:, lo:tcol], t_sb[:])
                            if tcol < ncol_t:
                                tacc = tpool.tile([H, TACC_COLS], f32,
                                                  tag="tacc")
            if tcol % TACC_COLS != 0:
                t_sb = opool.tile([H, TACC_COLS], f32, tag="tsb")
                rem = tcol % TACC_COLS
                nc.vector.tensor_copy(t_sb[:, :rem], tacc[:, :rem])
                nc.sync.dma_start(t_out.ap()[:, tcol - rem:tcol],
                                  t_sb[:, :rem])

            # ---------------- node phase ----------------
            sacc = spool.tile([H, ncol_s], f32)
            for b in range(B):
                at_sb = npool.tile([H, P, mn], dt_lo, tag="nat")
                nc.sync.dma_start(at_sb[:], na_in.ap()[b])
                zn = zpool.tile([H, mn], f32, tag="z")
                for p in range(P):
                    nc.tensor.matmul(
                        zn[:],
                        v_sb[:, b * P + p, :],
                        at_sb[:, p, :],
                        start=(p == 0), stop=(p == P - 1),
                    )
                en = npool.tile([H, mn], dt_lo, tag="en")
                nc.scalar.activation(en[:], zn[:], Exp)
                psn = npool.tile([H, mn], dt_lo, tag="psn")
                nc.vector._custom_dve(elu_op, out=psn[:], in0=zn[:], in1=en[:])
                for c in range(nchunk_s):
                    w = min(H, mn - c * H)
                    nc.tensor.matmul(
                        sacc[:w, b * nchunk_s + c: b * nchunk_s + c + 1],
                        psn[:, c * H:c * H + w],
                        wn_sb[:],
                        start=True, stop=True,
                    )
            s_sb = opool.tile([H, ncol_s], f32, tag="ssb")
            nc.vector.tensor_copy(s_sb[:], sacc[:])
            nc.sync.dma_start(s_out.ap()[:], s_sb[:])

    nc.compile()
    return nc


# ---------------------------------------------------------------------------
def kernel(node_attrs, edge_attrs, instruction_batch, distribution,
           node_prop_similarities, relation_similarity,
           W_props, W_edge, w_node_score, w_rel_score,
           edge_indices, node_indices, edge_batch_indices):
    _install_ntff_hook()
    from concourse import bass_utils

    np_lo = np.float16 if _DT == "fp16" else np.float32

    na = np.asarray(node_attrs, np.float32)
    ea = np.asarray(edge_attrs, np.float32)
    ib = np.asarray(instruction_batch, np.float32)
    dist = np.asarray(distribution, np.float32)
    nps = np.asarray(node_prop_similarities, np.float32)
    rs = np.asarray(relation_similarity, np.float32)
    Wp = np.asarray(W_props, np.float32)
    We = np.asarray(W_edge, np.float32)
    wn = np.asarray(w_node_score, np.float32)
    wr = np.asarray(w_rel_score, np.float32)
    ei = np.asarray(edge_indices).astype(np.int64)
    ni = np.asarray(node_indices).astype(np.int64)
    ebi = np.asarray(edge_batch_indices).astype(np.int64)
    src, dst = ei[0], ei[1]

    # ---- transformed weight tables (host, exact f32 then cast) ----
    U = ib[:, :, None] * We[None, :, :]                        # (B,H,H)
    V = (nps[:, :, None, None] * ib[:, None, :, None] *
         Wp[None, :, :, :])                                    # (B,P,H,H)
    U_lo = np.ascontiguousarray(U.transpose(1, 0, 2)).astype(np_lo)   # (H,B,H)
    V_lo = np.ascontiguousarray(
        V.reshape(B * P, H, H).transpose(1, 0, 2)).astype(np_lo)      # (H,B*P,H)
    wr_lo = wr.reshape(H, 1).astype(np_lo)
    wn_lo = wn.reshape(H, 1).astype(np_lo)

    # ---- edge sharding: sort by graph, pad groups, deal 8 ways ----
    order = np.argsort(ebi, kind="stable")
    ecounts = np.bincount(ebi, minlength=B)
    estarts = np.concatenate([[0], np.cumsum(ecounts)[:-1]])
    me = ((int(ecounts.max()) + NCORES * ZCOLS - 1)
          // (NCORES * ZCOLS)) * ZCOLS                         # per (core,b)
    assert me % ZCOLS == 0 and ecounts.max() <= NCORES * me
    ea_lo = ea.astype(np_lo)
    ebuf = np.zeros((B, NCORES * me, H), np_lo)
    for b in range(B):
        s, c = estarts[b], ecounts[b]
        ebuf[b, :c] = ea_lo[order[s:s + c]]
    # (B, cores, me, H) -> (cores, B, H, me)
    ea_t = np.ascontiguousarray(
        ebuf.reshape(B, NCORES, me, H).transpose(1, 0, 3, 2)
    )
    del ebuf

    # ---- node sharding: already sorted by graph ----
    ncounts = np.bincount(ni, minlength=B)
    nstarts = np.concatenate([[0], np.cumsum(ncounts)[:-1]])
    mn = ((int(ncounts.max()) + NCORES - 1) // NCORES + 1) // 2 * 2
    mn = min(max(mn, 2), TZ)
    assert ncounts.max() <= NCORES * mn
    na_lo = na.astype(np_lo)
    nbuf = np.zeros((B, NCORES * mn, P, H), np_lo)
    for b in range(B):
        s, c = nstarts[b], ncounts[b]
        nbuf[b, :c] = na_lo[s:s + c]
    # (B, cores, mn, P, H) -> (cores, B, H, P, mn)
    na_t = np.ascontiguousarray(
        nbuf.reshape(B, NCORES, mn, P, H).transpose(1, 0, 4, 3, 2)
    )
    del nbuf

    import concourse.mybir as mybir
    dt_lo = mybir.dt.float16 if np_lo is np.float16 else mybir.dt.float32

    nc = _build_program(me, mn, dt_lo, np_lo)

    in_maps = []
    for c in range(NCORES):
        in_maps.append({
            "ea_t": ea_t[c],
            "na_t": na_t[c],
            "u_tab": U_lo,
            "v_tab": V_lo,
            "w_rel": wr_lo,
            "w_node": wn_lo,
        })

    res = bass_utils.run_bass_kernel_spmd(
        nc, in_maps, core_ids=list(range(NCORES)),
        trace=bool(os.environ.get("BASS_TRACE")),
        tmpdir=os.environ.get("KERNEL_TRACE_DIR") or None,
    )
    kernel.last_results = res  # for test.py profiling introspection

    # ---- host epilogue ----
    # t mapping: t_dev[c][p, col], col = b*(me//H) + cc, edge pos in (c,b)
    # chunk = cc*H + p; global pos in sorted group = c*me + cc*H + p.
    ncol_t = B * me // H
    cpg = me // H                                    # cols per graph
    t_dev = np.stack([np.asarray(res.results[c]["t_out"])
                      for c in range(NCORES)])       # (8, H, ncol_t)
    col = np.arange(ncol_t)
    b_of_col = col // cpg
    cc_of_col = col % cpg
    # pos[c,p,col] = c*me + cc*H + p
    pos = (np.arange(NCORES)[:, None, None] * me
           + cc_of_col[None, None, :] * H
           + np.arange(H)[None, :, None])            # (8,H,ncol_t)
    bgrid = np.broadcast_to(b_of_col[None, None, :], pos.shape)
    valid = pos < ecounts[bgrid]
    sorted_idx = estarts[bgrid] + pos                # index into `order`
    t_full = np.zeros(E, np.float64)
    t_full[order[sorted_idx[valid]]] = t_dev[valid]

    # s mapping: col = b*nchunk_s + chunk; node pos = c*mn + chunk*H + p
    nchunk_s = (mn + H - 1) // H
    ncol_s = B * nchunk_s
    s_dev = np.stack([np.asarray(res.results[c]["s_out"])
                      for c in range(NCORES)])       # (8, H, ncol_s)
    scol = np.arange(ncol_s)
    b_of_scol = scol // nchunk_s
    ch_of_scol = scol % nchunk_s
    spos = (np.arange(NCORES)[:, None, None] * mn
            + ch_of_scol[None, None, :] * H
            + np.arange(H)[None, :, None])
    sbgrid = np.broadcast_to(b_of_scol[None, None, :], spos.shape)
    svalid = (spos < ncounts[sbgrid]) & ((ch_of_scol[None, None, :] * H
              + np.arange(H)[None, :, None]) < mn)
    s_full = np.zeros(N, np.float64)
    s_full[nstarts[sbgrid[svalid]] + spos[svalid]] = s_dev[svalid]

    # scatter-add edge scalars into nodes, then segment softmaxes
    acc = np.bincount(dst, weights=dist[src].astype(np.float64) * t_full,
                      minlength=N)

    def seg_softmax(x):
        m = np.full(B, -np.inf)
        np.maximum.at(m, ni, x)
        e = np.exp(x - m[ni])
        ssum = np.zeros(B, np.float64)
        np.add.at(ssum, ni, e)
        return e / ssum[ni]

    next_rel = seg_softmax(acc)
    next_states = seg_softmax(s_full)
    rsn = rs[ni].astype(np.float64)
    out = rsn * next_rel + (1.0 - rsn) * next_states
    return out.astype(np.float32)
